# revision 2
# baseline (speedup 1.0000x reference)
"""Trainium2 Bass kernel for nn_BornFoward: 200-step leapfrog wave recurrence.

12 matmuls/step (vs baseline 14), with all cross-step false dependencies
removed by using SEPARATE tensors per (buffer, chunk):

  - State: T[b][c] = sbuf [119, 300] fp32r; partitions 0..70 = p rows of
    chunk c (cols [4 | f0:2+142+2 | f1:2+142+2 | 4]); partitions 71..118 =
    G[j] for chunk c (host-padded full-width rows -> one contiguous DMA per
    chunk per step, 2-step prefetch lead).
  - PSUM: P[w][c] = [128, 292] (4 banks; w = step parity). Per chunk:
    M1a (K=119: band+2I+G-inject, + 32 meas-selection aug cols -> psum rows
    96..127, start=True), 4 diag shift matmuls (y-stencil), cross matmul
    (K=71, rhs = other chunk tile, lhsT nonzero only at 2 boundary rows,
    stop=True). Copyback (DVE TT psum - p0) right after each chunk's block.
  - Measurements: ACT copies psum sel rows [32, 292] per chunk into an SBUF
    ring half; one DMA per 4 steps flushes a full half to DRAM (SELO); host
    gathers the receiver pixels. No DVE measurement work.
  - Block order alternates per step so the first block of step j+1 only
    depends on the early copyback of step j.
"""
import sys
import os
import numpy as np
from contextlib import ExitStack

sys.path.insert(0, "/opt/trn_rl_repo")

# ---- problem constants ----
NX = 192
NT = 200
dtime = 0.3
nm, sR = 32, 70
bg = 1.5
LO, HI = 25, 167            # interior rows/cols [LO, HI) -> D = 142
D = HI - LO                 # 142
CLO, CHI = 48, 144          # central rf-support window (96 wide)
CW = CHI - CLO              # 96
COFF = CLO - LO             # 23
C = (dtime * bg / 1.0) ** 2  # 0.2025
K = 71                      # row-chunk size
SEG = 2 + D + 2             # 146
WIN = 2 * SEG               # 292 (one chunk window: 2 fields)
PAD = 4
TCOLS = PAD + WIN + PAD     # 300
NRR = 8
BB = 2
NMEAS = nm
NGROWS = 48                 # G rows per chunk
GP0 = K                     # G partitions start (71)
NPART = GP0 + NGROWS        # 119
M = 128                     # psum rows: 71 data + aug at 96..127
RING = 4                    # steps per ring half

_thetas = 2 * np.pi * np.arange(nm) / nm
_MX = (NX / 2 + sR * np.cos(_thetas)).astype(int)
_MY = (NX / 2 + sR * np.sin(_thetas)).astype(int)

_prog_cache = {}


def _band_matrix():
    """Full 142x142 interior operator S = 2I + C*Lx (x part + diag)."""
    S = np.zeros((D, D), np.float32)
    idx = np.arange(D)
    S[idx, idx] = -60.0 * C / 12.0 + 2.0
    S[idx[:-1], idx[:-1] + 1] = 16.0 * C / 12.0
    S[idx[1:], idx[1:] - 1] = 16.0 * C / 12.0
    S[idx[:-2], idx[:-2] + 2] = -C / 12.0
    S[idx[2:], idx[2:] - 2] = -C / 12.0
    return S


def _build_consts():
    S = _band_matrix()
    a = np.float32(16.0 * C / 12.0)
    b = np.float32(-C / 12.0)

    consts = {}
    for c in range(2):
        r0 = c * K
        big = np.zeros((NPART, M), np.float32)
        big[0:K, 0:K] = S[r0:r0 + K, r0:r0 + K].T
        for i in range(NGROWS):
            orow = (COFF + i) if c == 0 else i
            big[GP0 + i, orow] = 1.0
        for i in range(NMEAS):
            g = _MX[i] - LO
            if g // K == c:
                big[g % K, 96 + i] = 1.0
        consts[f"BIG{c}"] = big

        cross = np.zeros((K, M), np.float32)
        if c == 0:
            cross[:, 0:K] = S[r0:r0 + K, K:2 * K].T
        else:
            cross[:, 0:K] = S[r0:r0 + K, 0:K].T
        consts[f"CROSS{c}"] = cross

        augf = np.zeros((K, M), np.float32)
        for i in range(NMEAS):
            g = _MX[i] - LO
            if g // K == c:
                augf[g % K, 96 + i] = 1.0
        consts[f"AUGF{c}"] = augf

    consts["SHA"] = np.eye(K, dtype=np.float32) * a
    consts["SHB"] = np.eye(K, dtype=np.float32) * b
    consts["ZEROT"] = np.zeros((NPART, TCOLS), np.float32)
    return consts


def _build_program(nt=NT, reps=1):
    import concourse.bacc as bacc
    import concourse.tile as tile
    import concourse.mybir as mybir

    dt = mybir.dt
    nc = bacc.Bacc("TRN2", target_bir_lowering=False)

    # G[j] per chunk: (NT, 2, 48, 292) host-padded contiguous rows
    G_d = nc.dram_tensor("G", (NT, 2, NGROWS, WIN), dt.float32r,
                         kind="ExternalInput")
    BIG_d = {c: nc.dram_tensor(f"BIG{c}", (NPART, M), dt.float32r,
                               kind="ExternalInput") for c in range(2)}
    CROSS_d = {c: nc.dram_tensor(f"CROSS{c}", (K, M), dt.float32r,
                                 kind="ExternalInput") for c in range(2)}
    AUGF_d = {c: nc.dram_tensor(f"AUGF{c}", (K, M), dt.float32r,
                                kind="ExternalInput") for c in range(2)}
    SHA_d = nc.dram_tensor("SHA", (K, K), dt.float32r, kind="ExternalInput")
    SHB_d = nc.dram_tensor("SHB", (K, K), dt.float32r, kind="ExternalInput")
    ZT_d = nc.dram_tensor("ZEROT", (NPART, TCOLS), dt.float32r,
                          kind="ExternalInput")
    # SELO row t: sel rows of step t (chunk-major: [2, 292]); +1 = final
    SELO_d = nc.dram_tensor("SELO", (NT + 1, NMEAS, 2 * WIN), dt.float32,
                            kind="ExternalOutput")

    with tile.TileContext(nc) as tc, ExitStack() as ctx:
        def sbuf(name, shape, dty):
            return ctx.enter_context(nc.sbuf_tensor(name, shape, dty))

        T = [[sbuf(f"T{b}{c}", [NPART, TCOLS], dt.float32r)
              for c in range(2)] for b in range(4)]
        big_t = {c: sbuf(f"big{c}", [NPART, M], dt.float32r)
                 for c in range(2)}
        cross_t = {c: sbuf(f"cross{c}", [K, M], dt.float32r)
                   for c in range(2)}
        augf_t = {c: sbuf(f"augf{c}", [K, M], dt.float32r)
                  for c in range(2)}
        sha_t = sbuf("sha", [K, K], dt.float32r)
        shb_t = sbuf("shb", [K, K], dt.float32r)
        P = [[ctx.enter_context(
            nc.psum_tensor(f"P{w}{c}", [M, WIN], dt.float32))
            for c in range(2)] for w in range(4)]
        ring_t = [sbuf(f"ring{h}", [NMEAS, RING * 2 * WIN], dt.float32)
                  for h in range(2)]

        # ---- init ----
        for b in range(4):
            for c in range(2):
                nc.sync.dma_start(T[b][c][:], ZT_d[:])
        for c in range(2):
            nc.sync.dma_start(big_t[c][:], BIG_d[c][:])
            nc.sync.dma_start(cross_t[c][:], CROSS_d[c][:])
            nc.sync.dma_start(augf_t[c][:], AUGF_d[c][:])
        nc.sync.dma_start(sha_t[:], SHA_d[:])
        nc.sync.dma_start(shb_t[:], SHB_d[:])

        def g_dma(j):
            for c in range(2):
                nc.sync.dma_start(
                    T[j % 4][c][GP0:GP0 + NGROWS, PAD:PAD + WIN],
                    G_d[j, c])

        def sview(b, c, phi, off=0):
            return T[b][c][0:phi, PAD + off:PAD + off + WIN]

        def data_view(b, c, cast_f32=False):
            v = T[b][c][0:K, PAD:PAD + WIN]
            if cast_f32:
                v = v.bitcast(dt.float32)
            return v.rearrange("p (f x) -> p f x", x=SEG)[:, :, 2:2 + D]

        def sel_copy(pt, j, c):
            """ACT: psum sel rows -> ring half (j//RING)%2, slot j%RING."""
            h = (j // RING) % 2
            base = (j % RING) * 2 * WIN + c * WIN
            nc.scalar.copy(ring_t[h][:, base:base + WIN],
                           pt[96:96 + NMEAS, :])

        def ring_flush(j):
            """Flush the just-completed ring half to SELO rows j-RING+1..j."""
            h = (j // RING) % 2
            nc.sync.dma_start(
                SELO_d[j - RING + 1:j + 1].rearrange("s p q -> p s q"),
                ring_t[h][:].rearrange("p (s q) -> p s q", s=RING))

        for rep in range(reps):
            if rep > 0:
                for b in range(4):
                    for c in range(2):
                        nc.sync.dma_start(T[b][c][:], ZT_d[:])
            for q in range(4):
                g_dma(q)
            for j in range(nt):
                cb = j % 4            # holds p_j (+ G[j])
                pb = (j + 1) % 4      # overwritten with p_{j+1}
                qb = (j + 3) % 4      # holds p_{j-1}
                w = j % 4

                for c in (0, 1):
                    pt = P[w][c]
                    full = pt[0:M, :]
                    d = pt[0:K, :]
                    nc.tensor.matmul(full, big_t[c][:], sview(cb, c, NPART),
                                     start=True, stop=False)
                    nc.tensor.matmul(d, sha_t[:], sview(cb, c, K, -1),
                                     start=False, stop=False)
                    nc.tensor.matmul(d, sha_t[:], sview(cb, c, K, 1),
                                     start=False, stop=False)
                    nc.tensor.matmul(d, shb_t[:], sview(cb, c, K, -2),
                                     start=False, stop=False)
                    nc.tensor.matmul(d, shb_t[:], sview(cb, c, K, 2),
                                     start=False, stop=False)
                    nc.tensor.matmul(full, cross_t[c][:], sview(cb, 1 - c, K),
                                     start=False, stop=True)
                    pd = pt[0:K, :].rearrange(
                        "p (f x) -> p f x", x=SEG)[:, :, 2:2 + D]
                    nc.vector.tensor_tensor(
                        out=data_view(pb, c), in0=pd,
                        in1=data_view(qb, c, cast_f32=True),
                        op=mybir.AluOpType.subtract)
                    sel_copy(pt, j, c)

                if j % RING == RING - 1:
                    ring_flush(j)
                if j + 4 < nt:
                    g_dma(j + 4)

            # final measurement (output nt-1): aug-only matmuls on the final
            # state, staged through ring slot 0 of half (nt//RING)%2
            fb = nt % 4
            h = (nt // RING) % 2
            for c in range(2):
                pt = P[fb % 4][c]
                nc.tensor.matmul(pt[0:M, :], augf_t[c][:], sview(fb, c, K),
                                 start=True, stop=True)
                nc.scalar.copy(ring_t[h][:, c * WIN:(c + 1) * WIN],
                               pt[96:96 + NMEAS, :])
            nc.sync.dma_start(SELO_d[nt], ring_t[h][:, 0:2 * WIN])

    nc.compile()
    return nc


def _host_prep(x, P0):
    """rf * d2(P0) central window, padded into per-chunk contiguous rows
    (NT, 2, 48, 292) per source channel."""
    xx = bg / x[:, 0]
    rf = (1.0 - xx * xx).astype(np.float32)           # (B, 96, 96)
    P0c = P0[0, :, :, CLO:CHI, CLO:CHI]               # (NR, NT, 96, 96)
    d2 = np.zeros_like(P0c)
    d2[:, 2:] = P0c[:, 2:] - 2.0 * P0c[:, 1:-1] + P0c[:, :-2]

    gs = []
    for r in range(NRR):
        G = (rf[None, :, :, :] * d2[r][:, None, :, :]).astype(np.float32)
        Gp = np.zeros((NT, 2, NGROWS, WIN), np.float32)
        for c in range(2):
            src = G[:, :, c * NGROWS:(c + 1) * NGROWS, :]  # (NT,B,48,96)
            for f in range(2):
                o = f * SEG + 2 + COFF
                Gp[:, c, :, o:o + CW] = src[:, f]
        gs.append(np.ascontiguousarray(Gp))
    return gs


def kernel(x, P0):
    x = np.asarray(x, dtype=np.float32)
    P0 = np.asarray(P0, dtype=np.float32)
    from concourse.bass_utils import run_bass_kernel_spmd

    if "prog" not in _prog_cache:
        _prog_cache["prog"] = _build_program()
    nc = _prog_cache["prog"]

    consts = _build_consts()
    gs = _host_prep(x, P0)

    in_maps = []
    for r in range(NRR):
        m = dict(consts)
        m["G"] = gs[r]
        in_maps.append(m)

    trace = bool(int(os.environ.get("KERNEL_TRACE", "0")))
    res = run_bass_kernel_spmd(nc, in_maps, core_ids=list(range(NRR)),
                               trace=trace)
    _prog_cache["last_result"] = res

    # host-side gather: output t <- SELO[t+1] (sel of step t+1 = p_{t+1});
    # receiver i at col (c_i*292 + f*146 + 2 + (my_i - LO))
    ci = (_MX - LO) // K
    out = np.zeros((BB, NRR, NMEAS, NT), np.float32)
    for r in range(NRR):
        selo = res.results[r]["SELO"]                 # (NT+1, 32, 584)
        for f in range(BB):
            cols = ci * WIN + f * SEG + 2 + (_MY - LO)
            out[f, r] = selo[1:, np.arange(NMEAS), cols].T
    return out
